# revision 2
# baseline (speedup 1.0000x reference)
"""Distributed Bass kernel for sparse cluster attention on 8 TRN2 NeuronCores.

Sharding: tensor-parallel over heads (16 heads -> 2 per core).
Per core:
  1. fp32-accurate keyframe q/k (hi/lo bf16 split) -> attn_score partial,
     AllReduce(max) over cores.
  2. main qkv in bf16: qT [ch,tok] in SBUF; k,v kept in SBUF as [tok,ch]
     tiles (no DRAM staging).
  3. on-device top-153 per cluster: rank via comparison matrix; selection
     materialized as per-cluster one-hot matrices [tok_chunk, rank] (bf16).
  4. gather k (-> [ch, j]) and v (-> packed [j, ch]) via one-hot matmuls on
     the tensor engine; flash-style attention (logits MM -> exp on ACT ->
     AV MM with ones-augmented v for the softmax denominator). Gathered
     blocks are packed 153*frames per source cluster, so every consumer
     cluster's kv list is a per-src prefix; partial chunks are handled by
     partition-range slices.
  5. AllToAll of per-core attention output -> proj on this core's token
     slice -> out [2048, 1024] f32; host concatenates.
"""

import numpy as np
import ml_dtypes

import os
import concourse.bass as bass
import concourse.bacc as bacc
import concourse.mybir as mybir
import concourse.tile as tile
from concourse.bass_utils import run_bass_kernel_spmd

BF16 = mybir.dt.bfloat16
F32 = mybir.dt.float32
I32 = mybir.dt.int32
AF = mybir.ActivationFunctionType
OP = mybir.AluOpType

# problem constants
H, D, C = 16, 64, 1024
S, P = 32, 512
K, FC = 4, 8
N = S * P                      # 16384 tokens
TK = 153                       # top-k patches per cluster
NSUB = 5                       # subsampled frames
NCORES = 8
HC = H // NCORES               # heads per core = 2
CHC = HC * D                   # channels per core = 128
TOKS = N // NCORES             # output tokens per core = 2048
SCALE = float(D) ** -0.5
KFT = K * P                    # keyframe tokens = 2048
FULL = FC * TK                 # packed kv rows per full src block = 1224
PRE5 = NSUB * TK               # packed kv rows per 5-frame prefix = 765
KGW = 1280                     # kg tile width (>= FULL, mult of 128)

_CACHE: dict = {}


def _chunks_for(ci):
    """(src, chunk, rows) list for consumer cluster ci over packed kv."""
    out = []
    for src in range(K):
        valid = FULL if src in (0, ci) else PRE5
        nch = (valid + 127) // 128
        for c in range(nch):
            out.append((src, c, min(128, valid - c * 128)))
    return out


def _v_pieces():
    """Per (chunk c): list of (frame f, p0, p1, j0) partition/col splits of
    the packed 153-per-frame v layout."""
    pieces = [[] for _ in range((FULL + 127) // 128)]
    for f in range(FC):
        r0, r1 = f * TK, (f + 1) * TK
        c = r0 // 128
        while c * 128 < r1:
            lo = max(r0, c * 128)
            hi = min(r1, (c + 1) * 128)
            pieces[c].append((f, lo - c * 128, hi - c * 128, lo - r0))
            c += 1
    return pieces


def build_nc(clusters, keyframes):
    NOAR = os.environ.get("KNOAR", "0") == "1"
    NOA2A = os.environ.get("KNOA2A", "0") == "1"
    STUB = os.environ.get("KSTUB", "0") == "1"
    nc = bacc.Bacc(None, target_bir_lowering=False, debug=False)

    # ---- kernel I/O (per-core shards prepared on host) ----
    xbT = nc.dram_tensor("xbT", [C, N], BF16, kind="ExternalInput")
    xkfT_l = nc.dram_tensor("xkfT_l", [C, KFT], BF16, kind="ExternalInput")
    wqkv = nc.dram_tensor("wqkv", [C, 3 * CHC], BF16, kind="ExternalInput")
    bqkv = nc.dram_tensor("bqkv", [3 * CHC], F32, kind="ExternalInput")
    wqk_h = nc.dram_tensor("wqk_h", [C, 2 * CHC], BF16, kind="ExternalInput")
    wqk_l = nc.dram_tensor("wqk_l", [C, 2 * CHC], BF16, kind="ExternalInput")
    bqk = nc.dram_tensor("bqk", [2 * CHC], F32, kind="ExternalInput")
    wproj = nc.dram_tensor("wproj", [C, C], BF16, kind="ExternalInput")
    bproj = nc.dram_tensor("bproj", [C], F32, kind="ExternalInput")
    out_ext = nc.dram_tensor("out", [TOKS, C], F32, kind="ExternalOutput")

    # ---- internal DRAM ----
    sc_in = nc.dram_tensor("sc_in", [K * P], F32)
    sc_out = nc.dram_tensor("sc_out", [K * P], F32, addr_space="Shared")
    ag_in = nc.dram_tensor("ag_in", [NCORES, CHC, TOKS], BF16)
    ag_out = nc.dram_tensor("ag_out", [NCORES, CHC, TOKS], BF16)

    if STUB:
        with tile.TileContext(nc) as tc:
            with tc.tile_pool(name="sp", bufs=2) as sp:
                t = sp.tile([128, 512], BF16)
                nc.sync.dma_start(t[:], xbT.ap()[0:128, 0:512])
                t2 = sp.tile([128, 512], F32)
                nc.vector.tensor_copy(t2[:], t[:])
                nc.sync.dma_start(out_ext.ap()[0:128, 0:512], t2[:])
        nc.finalize()
        return nc

    with tile.TileContext(nc) as tc:
        with (
            tc.tile_pool(name="persist", bufs=1) as pp,
            tc.tile_pool(name="work", bufs=3) as wp,
            tc.tile_pool(name="xp", bufs=16) as xp,
            tc.tile_pool(name="expw", bufs=2) as ep,
            tc.tile_pool(name="psmed", bufs=2, space="PSUM") as psM,
            tc.tile_pool(name="psav", bufs=2, space="PSUM") as psV,
            tc.tile_pool(name="psbig", bufs=1, space="PSUM") as psL,
        ):
            # ================= persistent SBUF =================
            qT = pp.tile([CHC, N], BF16, tag="qT")               # 4 MB
            k_sb = pp.tile([128, N // 128, CHC], BF16, tag="ksb")  # 4 MB
            v_sb = pp.tile([128, N // 128, CHC], BF16, tag="vsb")  # 4 MB
            kg = pp.tile([128, K, KGW], BF16, tag="kg")          # 1.25 MB
            vaug = pp.tile([128, K, 10, 130], BF16, tag="vaug")  # 1.3 MB
            oh = pp.tile([128, K * 4, 160], BF16, tag="oh")
            ones_rowb = pp.tile([1, 128], BF16, tag="onesb")
            nc.vector.memset(ones_rowb[:], 1.0)
            onesf_row = pp.tile([1, 128], F32, tag="onesf")
            nc.vector.memset(onesf_row[:], 1.0)
            onesf_col = pp.tile([128, 1], F32, tag="onesfc")
            nc.vector.memset(onesf_col[:], 1.0)

            # weight tiles
            wqkv_t = pp.tile([128, 8, 3 * CHC], BF16, tag="wqkv")
            nc.sync.dma_start(wqkv_t[:], wqkv.ap().rearrange("(a p) c -> p a c", p=128))
            wqkh_t = pp.tile([128, 8, 2 * CHC], BF16, tag="wqkh")
            nc.sync.dma_start(wqkh_t[:], wqk_h.ap().rearrange("(a p) c -> p a c", p=128))
            wqkl_t = pp.tile([128, 8, 2 * CHC], BF16, tag="wqkl")
            nc.sync.dma_start(wqkl_t[:], wqk_l.ap().rearrange("(a p) c -> p a c", p=128))

            # bias columns (per-partition layout)
            bq_col = pp.tile([128, 1], F32, tag="bqcol")
            nc.sync.dma_start(bq_col[:], bqkv.ap()[0:CHC].rearrange("(p a) -> p a", a=1))
            bkv_row = pp.tile([1, 2 * CHC], F32, tag="bkvrow")
            nc.sync.dma_start(bkv_row[:], bqkv.ap()[CHC:3 * CHC].rearrange("(a c) -> a c", a=1))
            bkv_row_b = pp.tile([1, 2 * CHC], BF16, tag="bkvrowb")
            nc.vector.tensor_copy(bkv_row_b[:], bkv_row[:])
            bqk_q = pp.tile([128, 1], F32, tag="bqkq")
            nc.sync.dma_start(bqk_q[:], bqk.ap()[0:CHC].rearrange("(p a) -> p a", a=1))
            bqk_k = pp.tile([128, 1], F32, tag="bqkk")
            nc.sync.dma_start(bqk_k[:], bqk.ap()[CHC:2 * CHC].rearrange("(p a) -> p a", a=1))

            # ================= phase 1: keyframe scores (fp32 accurate) ======
            qkf = pp.tile([128, KFT], F32, tag="qkf")
            kkf = pp.tile([128, KFT], F32, tag="kkf")
            for tt in range(KFT // 512):
                kf = int(keyframes[tt])
                xh = [xp.tile([128, 512], BF16, tag="xmain", name=f"xh{tt}_{i}") for i in range(8)]
                xl = [xp.tile([128, 512], BF16, tag="xmain", name=f"xl{tt}_{i}") for i in range(8)]
                for cc in range(8):
                    nc.sync.dma_start(xh[cc][:], xbT.ap()[cc * 128:(cc + 1) * 128, kf * 512:(kf + 1) * 512])
                    nc.sync.dma_start(xl[cc][:], xkfT_l.ap()[cc * 128:(cc + 1) * 128, tt * 512:(tt + 1) * 512])
                for ot, (dst, bias) in enumerate(((qkf, bqk_q), (kkf, bqk_k))):
                    ps = psM.tile([128, 512], F32, tag="med")
                    nmm = 8 * 3
                    i = 0
                    for cc in range(8):
                        w_h = wqkh_t[:, cc, ot * CHC:(ot + 1) * CHC]
                        w_l = wqkl_t[:, cc, ot * CHC:(ot + 1) * CHC]
                        for (wt, xt) in ((w_h, xh[cc]), (w_h, xl[cc]), (w_l, xh[cc])):
                            nc.tensor.matmul(ps[:], wt, xt[:], start=(i == 0), stop=(i == nmm - 1))
                            i += 1
                    nc.vector.tensor_scalar(dst[:, tt * 512:(tt + 1) * 512], ps[:], bias[:], None, OP.add)

            # prod + per-head reduce + max over the 2 local heads
            nc.vector.tensor_tensor(qkf[:], qkf[:], kkf[:], OP.mult)  # qkf <- q*k
            for ntile in range(KFT // 512):
                sl = slice(ntile * 512, (ntile + 1) * 512)
                ps0 = psM.tile([1, 512], F32, tag="med")
                ps1 = psM.tile([1, 512], F32, tag="med")
                nc.tensor.matmul(ps0[:], onesf_col[0:64, :], qkf[0:64, sl], start=True, stop=True)
                nc.tensor.matmul(ps1[:], onesf_col[64:128, :], qkf[64:128, sl], start=True, stop=True)
                s1sb = wp.tile([1, 512], F32, tag="s1sb", bufs=1)
                nc.vector.tensor_copy(s1sb[:], ps1[:])
                smax_t = wp.tile([1, 512], F32, tag="smax", bufs=2)
                nc.vector.tensor_tensor(smax_t[:], ps0[:], s1sb[:], OP.max)
                nc.sync.dma_start(
                    sc_in.ap()[ntile * 512:(ntile + 1) * 512].rearrange("(a c) -> a c", a=1),
                    smax_t[:])
            if NOAR:
                nc.sync.dma_start(sc_out.ap(), sc_in.ap())
            else:
                nc.gpsimd.collective_compute(
                    "AllReduce", OP.max,
                    replica_groups=[list(range(NCORES))],
                    ins=[sc_in.ap().opt()],
                    outs=[sc_out.ap().opt()],
                )

            # ================= phase 2: main qkv (bf16) =================
            for tt in range(N // 512):
                xt = [xp.tile([128, 512], BF16, tag="xmain", name=f"xt{tt}_{i}") for i in range(8)]
                for cc in range(8):
                    nc.sync.dma_start(xt[cc][:], xbT.ap()[cc * 128:(cc + 1) * 128, tt * 512:(tt + 1) * 512])
                # q: [ch, tok]
                psq = psM.tile([128, 512], F32, tag="med")
                for cc in range(8):
                    nc.tensor.matmul(psq[:], wqkv_t[:, cc, 0:CHC], xt[cc][:],
                                     start=(cc == 0), stop=(cc == 7))
                nc.vector.tensor_scalar(qT[:, tt * 512:(tt + 1) * 512], psq[:], bq_col[:], None, OP.add)
                # k,v: [tok, ch] kept in SBUF
                for sub in range(4):
                    pskv = psM.tile([128, 2 * CHC], F32, tag="med")
                    for cc in range(8):
                        nc.tensor.matmul(pskv[:], xt[cc][:, sub * 128:(sub + 1) * 128],
                                         wqkv_t[:, cc, CHC:3 * CHC],
                                         start=(cc == 0), stop=False)
                    nc.tensor.matmul(pskv[:], ones_rowb[:], bkv_row_b[:],
                                     start=False, stop=True)
                    nc.vector.tensor_copy(k_sb[:, tt * 4 + sub, :], pskv[:, 0:CHC])
                    nc.vector.tensor_copy(v_sb[:, tt * 4 + sub, :], pskv[:, CHC:2 * CHC])

            # ================= phase 3: top-k -> one-hot selection ===========
            iota160 = wp.tile([128, 160], I32, tag="io160")
            nc.gpsimd.iota(iota160[:], pattern=[[1, 160]], base=0, channel_multiplier=0)
            iota160f = pp.tile([128, 160], F32, tag="io160f")
            nc.vector.tensor_copy(iota160f[:], iota160[:])
            iota_pv = wp.tile([128, 4], I32, tag="iopv")
            nc.gpsimd.iota(iota_pv[:], pattern=[[128, 4]], base=0, channel_multiplier=1)
            iota_pvf = pp.tile([128, 4], F32, tag="iopvf")
            nc.vector.tensor_copy(iota_pvf[:], iota_pv[:])

            for cl in range(K):
                s_row = wp.tile([1, P], F32, tag="srow")
                nc.sync.dma_start(s_row[:], sc_out.ap()[cl * P:(cl + 1) * P].rearrange("(a c) -> a c", a=1))
                s_colT = wp.tile([128, 4], F32, tag="scolT")
                nc.sync.dma_start(
                    s_colT[:], sc_out.ap()[cl * P:(cl + 1) * P].rearrange("(a p) -> p a", p=128))
                ps_bc = psM.tile([128, P], F32, tag="med")
                nc.tensor.matmul(ps_bc[:], onesf_row[:], s_row[:], start=True, stop=True)
                s_bc = wp.tile([128, P], F32, tag="sbc", bufs=2)
                nc.vector.tensor_copy(s_bc[:], ps_bc[:])
                ps_row = psM.tile([1, 160], F32, tag="med")
                for pc in range(4):
                    gt = wp.tile([128, P], F32, tag="gtm", bufs=2)
                    nc.vector.tensor_scalar(gt[:], s_bc[:], s_colT[:, pc:pc + 1], None, OP.is_gt)
                    rank = wp.tile([128, 1], F32, tag="rank")
                    nc.vector.reduce_sum(rank[:], gt[:], axis=mybir.AxisListType.X)
                    eqr = wp.tile([128, 160], F32, tag="eqr", bufs=2)
                    nc.vector.tensor_scalar(eqr[:], iota160f[:], rank[:], None, OP.is_equal)
                    nc.tensor.matmul(ps_row[:], iota_pvf[:, pc:pc + 1], eqr[:],
                                     start=(pc == 0), stop=(pc == 3))
                psel_row = wp.tile([1, 160], F32, tag="pselr")
                nc.vector.tensor_copy(psel_row[:], ps_row[:])
                ps_b = psM.tile([128, 160], F32, tag="med")
                nc.tensor.matmul(ps_b[:], onesf_row[:], psel_row[:], start=True, stop=True)
                psB = wp.tile([128, 160], F32, tag="psB", bufs=2)
                nc.vector.tensor_copy(psB[:], ps_b[:])
                for i in range(4):
                    iota_c = wp.tile([128, 1], I32, tag="ioc", name=f"ioc{cl}_{i}")
                    nc.gpsimd.iota(iota_c[:], pattern=[[0, 1]], base=128 * i, channel_multiplier=1)
                    iota_cf = wp.tile([128, 1], F32, tag="iocf", name=f"iocf{cl}_{i}")
                    nc.vector.tensor_copy(iota_cf[:], iota_c[:])
                    nc.vector.tensor_scalar(oh[:, cl * 4 + i, :], psB[:], iota_cf[:], None, OP.is_equal)

            # ================= phase 3b: one-hot matmul gathers ==============
            # K: kg[ch, src, f*153 + rank] = k[token(frame f, patch rank), ch]
            for src in range(K):
                for f8 in range(FC):
                    fr = int(clusters[src][f8])
                    psk = psM.tile([128, 512], F32, tag="med")
                    for i in range(4):
                        nc.tensor.matmul(psk[:, 0:160], k_sb[:, fr * 4 + i, :],
                                         oh[:, src * 4 + i, :],
                                         start=(i == 0), stop=(i == 3))
                    nc.vector.tensor_copy(kg[:, src, f8 * TK:(f8 + 1) * TK], psk[:, 0:TK])
            # V: packed [j, ch] with ones-augmentation per head
            vpieces = _v_pieces()
            for src in range(K):
                for c in range(10):
                    psv = psM.tile([128, 512], F32, tag="med")
                    for (f8, p0, p1, j0) in vpieces[c]:
                        fr = int(clusters[src][f8])
                        w = p1 - p0
                        for i in range(4):
                            nc.tensor.matmul(psv[p0:p1, 0:CHC],
                                             oh[:, src * 4 + i, j0:j0 + w],
                                             v_sb[:, fr * 4 + i, :],
                                             start=(i == 0), stop=(i == 3))
                    nc.vector.tensor_copy(vaug[:, src, c, 0:64], psv[:, 0:64])
                    nc.vector.tensor_copy(vaug[:, src, c, 65:129], psv[:, 64:CHC])
            nc.vector.memset(vaug[:, :, :, 64:65], 1.0)
            nc.vector.memset(vaug[:, :, :, 129:130], 1.0)

            # ================= phase 4: attention per cluster =================
            for ci in range(K):
                chunks = _chunks_for(ci)
                ngrp = len(chunks) // 2
                for qt in range(FC):
                    f_q = int(clusters[ci][qt])
                    qsl = slice(f_q * P, (f_q + 1) * P)
                    ps_av = [psV.tile([65, 512], F32, tag="av", name=f"psav{ci}_{qt}_{i}") for i in range(2)]
                    for g in range(ngrp):
                        ps_lg = psL.tile([128, 2048], F32, tag="big")
                        for k01 in range(2):
                            src, c, rows = chunks[g * 2 + k01]
                            for h in range(2):
                                nc.tensor.matmul(
                                    ps_lg[:, (k01 * 2 + h) * 512:(k01 * 2 + h + 1) * 512],
                                    kg[h * 64:(h + 1) * 64, src, c * 128:(c + 1) * 128],
                                    qT[h * 64:(h + 1) * 64, qsl],
                                    start=True, stop=True,
                                    tile_position=(h * 64, 0))
                        ew = ep.tile([128, 2048], BF16, tag="ew")
                        nc.scalar.activation(ew[:], ps_lg[:], AF.Exp, scale=SCALE)
                        for k01 in range(2):
                            src, c, rows = chunks[g * 2 + k01]
                            for h in range(2):
                                nc.tensor.matmul(
                                    ps_av[h][:],
                                    vaug[0:rows, src, c, h * 65:(h + 1) * 65],
                                    ew[0:rows, (k01 * 2 + h) * 512:(k01 * 2 + h + 1) * 512],
                                    start=(g == 0 and k01 == 0),
                                    stop=(g == ngrp - 1 and k01 == 1))
                    # normalize and ship straight to the AllToAll staging buffer
                    otile = ep.tile([128, 512], BF16, tag="ot")
                    for h in range(2):
                        rec = wp.tile([1, 512], F32, tag="rec")
                        nc.vector.reciprocal(rec[:], ps_av[h][64:65, :])
                        ps_bc2 = psM.tile([64, 512], F32, tag="med")
                        nc.tensor.matmul(ps_bc2[:], onesf_row[:, 0:64], rec[:],
                                         start=True, stop=True)
                        bc_sb = wp.tile([64, 512], F32, tag="bcsb", bufs=2)
                        nc.vector.tensor_copy(bc_sb[:], ps_bc2[:])
                        nc.vector.tensor_tensor(
                            otile[h * 64:(h + 1) * 64, :],
                            ps_av[h][0:64, :], bc_sb[:], OP.mult)
                    jcore = (f_q * P) // TOKS
                    toff = (f_q * P) % TOKS
                    nc.sync.dma_start(ag_in.ap()[jcore, :, toff:toff + 512], otile[:])

            # ================= phase 5: AllToAll + proj =================
            if NOA2A:
                nc.sync.dma_start(ag_out.ap(), ag_in.ap())
            else:
                nc.gpsimd.collective_compute(
                    "AllToAll", OP.bypass,
                    replica_groups=[list(range(NCORES))],
                    ins=[ag_in.ap().opt()],
                    outs=[ag_out.ap().opt()],
                )
            wpj = pp.tile([128, 8, C], BF16, tag="wpj")
            nc.sync.dma_start(wpj[:], wproj.ap().rearrange("(a p) c -> p a c", p=128))
            bpj_row = pp.tile([1, C], BF16, tag="bpj")
            bpj_f = wp.tile([1, C], F32, tag="bpjf")
            nc.sync.dma_start(bpj_f[:], bproj.ap().rearrange("(a c) -> a c", a=1))
            nc.vector.tensor_copy(bpj_row[:], bpj_f[:])
            atk2 = pp.tile([128, 8, TOKS], BF16, tag="atk2")
            nc.sync.dma_start(atk2[:], ag_out.ap().rearrange("j p t -> p j t"))
            for mt in range(TOKS // 128):
                for ntile in range(2):
                    nsl = slice(ntile * 512, (ntile + 1) * 512)
                    ps = psM.tile([128, 512], F32, tag="med")
                    for cc in range(8):
                        nc.tensor.matmul(ps[:], atk2[:, cc, mt * 128:(mt + 1) * 128],
                                         wpj[:, cc, nsl], start=(cc == 0), stop=False)
                    nc.tensor.matmul(ps[:], ones_rowb[:], bpj_row[:, nsl],
                                     start=False, stop=True)
                    ot = wp.tile([128, 512], F32, tag="otile", bufs=2)
                    nc.vector.tensor_copy(ot[:], ps[:])
                    nc.sync.dma_start(
                        out_ext.ap()[mt * 128:(mt + 1) * 128, nsl], ot[:])

    nc.finalize()
    return nc


def _host_prep(x, W_qkv, b_qkv, W_proj, b_proj, clusters, keyframes):
    bf = ml_dtypes.bfloat16
    x2 = np.ascontiguousarray(x.reshape(N, C))
    xbT = np.ascontiguousarray(x2.T.astype(bf))                       # [C, N]
    kf_tok = np.concatenate([np.arange(P, dtype=np.int64) + int(f) * P for f in keyframes])
    xkf = x2[kf_tok]                                                   # [2048, C] f32
    xkf_h = xkf.astype(bf)
    xkf_l = (xkf - xkf_h.astype(np.float32)).astype(bf)
    xkfT_l = np.ascontiguousarray(xkf_l.T)

    in_maps = []
    for core in range(NCORES):
        h0 = core * HC
        qcols = np.arange(h0 * D, (h0 + HC) * D)
        wq = W_qkv[:, qcols]
        wk = W_qkv[:, C + qcols]
        wv = W_qkv[:, 2 * C + qcols]
        wqkv_s = np.concatenate([wq, wk, wv], axis=1)                  # [C, 384]
        bq = b_qkv[qcols]
        bk = b_qkv[C + qcols]
        bv = b_qkv[2 * C + qcols]
        wqk = np.concatenate([wq, wk], axis=1)                         # [C, 256]
        wqk_hi = wqk.astype(bf)
        wqk_lo = (wqk - wqk_hi.astype(np.float32)).astype(bf)
        in_maps.append({
            "xbT": xbT,
            "xkfT_l": xkfT_l,
            "wqkv": np.ascontiguousarray(wqkv_s.astype(bf)),
            "bqkv": np.ascontiguousarray(np.concatenate([bq, bk, bv]).astype(np.float32)),
            "wqk_h": np.ascontiguousarray(wqk_hi),
            "wqk_l": np.ascontiguousarray(wqk_lo),
            "bqk": np.ascontiguousarray(np.concatenate([bq, bk]).astype(np.float32)),
            "wproj": np.ascontiguousarray(W_proj.astype(bf)),
            "bproj": np.ascontiguousarray(b_proj.astype(np.float32)),
        })
    return in_maps


def kernel(x, W_qkv, b_qkv, W_proj, b_proj, clusters, keyframes, **run_kwargs):
    x = np.asarray(x, dtype=np.float32)
    W_qkv = np.asarray(W_qkv, dtype=np.float32)
    b_qkv = np.asarray(b_qkv, dtype=np.float32)
    W_proj = np.asarray(W_proj, dtype=np.float32)
    b_proj = np.asarray(b_proj, dtype=np.float32)
    clusters = np.asarray(clusters, dtype=np.int32)
    keyframes = np.asarray(keyframes, dtype=np.int32)

    key = (clusters.tobytes(), keyframes.tobytes(),
           os.environ.get("KNOAR"), os.environ.get("KNOA2A"), os.environ.get("KSTUB"))
    if _CACHE.get("key") != key:
        _CACHE["nc"] = build_nc(clusters, keyframes)
        _CACHE["key"] = key
    nc = _CACHE["nc"]

    in_maps = _host_prep(x, W_qkv, b_qkv, W_proj, b_proj, clusters, keyframes)
    res = run_bass_kernel_spmd(nc, in_maps, core_ids=list(range(NCORES)), **run_kwargs)
    _CACHE["last_result"] = res
    outs = res.results
    full = np.concatenate([np.asarray(outs[c]["out"], dtype=np.float32) for c in range(NCORES)], axis=0)
    return full.reshape(1, N, C)


def bench(x, W_qkv, b_qkv, W_proj, b_proj, clusters, keyframes, iters=10, reps=5):
    """Steady-state on-device timing: times the best of `reps` calls."""
    import time
    import jax
    from jax.sharding import Mesh, PartitionSpec
    from jax.experimental.shard_map import shard_map
    from concourse import bass2jax
    from concourse.bass2jax import _bass_exec_p
    import concourse.mybir as _mb

    clusters = np.asarray(clusters, dtype=np.int32)
    keyframes = np.asarray(keyframes, dtype=np.int32)
    key = (clusters.tobytes(), keyframes.tobytes(),
           os.environ.get("KNOAR"), os.environ.get("KNOA2A"), os.environ.get("KSTUB"))
    if _CACHE.get("key") != key:
        _CACHE["nc"] = build_nc(clusters, keyframes)
        _CACHE["key"] = key
    nc = _CACHE["nc"]
    bass2jax.install_neuronx_cc_hook()

    in_maps = _host_prep(np.asarray(x, np.float32), np.asarray(W_qkv, np.float32),
                         np.asarray(b_qkv, np.float32), np.asarray(W_proj, np.float32),
                         np.asarray(b_proj, np.float32), clusters, keyframes)

    in_names, out_names, out_avals, zero_outs = [], [], [], []
    partition_name = nc.partition_id_tensor.name if nc.partition_id_tensor else None
    for alloc in nc.m.functions[0].allocations:
        if not isinstance(alloc, _mb.MemoryLocationSet):
            continue
        name = alloc.memorylocations[0].name
        if alloc.kind == "ExternalInput":
            if name != partition_name:
                in_names.append(name)
        elif alloc.kind == "ExternalOutput":
            out_names.append(name)
            shape = tuple(alloc.tensor_shape)
            dtype = _mb.dt.np(alloc.dtype)
            out_avals.append(jax.core.ShapedArray(shape, dtype))
            zero_outs.append(np.zeros(shape, dtype))
    n_params = len(in_names)
    all_in_names = list(in_names) + list(out_names)
    if partition_name is not None:
        all_in_names.append(partition_name)

    def _body(*args):
        ops = list(args)
        if partition_name is not None:
            ops = ops + [bass2jax.partition_id_tensor()]
        outs = _bass_exec_p.bind(
            *ops,
            out_avals=tuple(out_avals),
            in_names=tuple(all_in_names),
            out_names=tuple(out_names),
            lowering_input_output_aliases=(),
            sim_require_finite=True,
            sim_require_nnan=True,
            nc=nc,
        )
        return tuple(outs)

    devices = jax.devices()[:NCORES]
    mesh = Mesh(np.asarray(devices), ("core",))
    in_specs = (PartitionSpec("core"),) * (n_params + len(out_names))
    out_specs = (PartitionSpec("core"),) * len(out_names)
    f = jax.jit(shard_map(_body, mesh=mesh, in_specs=in_specs,
                          out_specs=out_specs, check_rep=False))
    concat_in = [np.concatenate([np.asarray(in_maps[c][n]) for c in range(NCORES)], axis=0)
                 for n in in_names]
    concat_zeros = [np.zeros((NCORES * z.shape[0], *z.shape[1:]), z.dtype) for z in zero_outs]
    args = [jax.device_put(a) for a in concat_in + concat_zeros]
    o = f(*args)
    jax.block_until_ready(o)
    times = []
    for _ in range(max(reps, 20)):
        t0 = time.perf_counter()
        o = f(*args)
        jax.block_until_ready(o)
        times.append(time.perf_counter() - t0)
    times.sort()
    return times[0] * 1e9, times


def bench_floor(reps=20):
    """Dispatch-floor: time a trivial 8-core NEFF (one 64KB copy)."""
    import time
    import jax
    from jax.sharding import Mesh, PartitionSpec
    from jax.experimental.shard_map import shard_map
    from concourse import bass2jax
    from concourse.bass2jax import _bass_exec_p
    import concourse.bacc as _bacc
    import concourse.tile as _tile

    if "floor_nc" not in _CACHE:
        nc = _bacc.Bacc(None, target_bir_lowering=False, debug=False)
        a = nc.dram_tensor("a", [128, 128], F32, kind="ExternalInput")
        b = nc.dram_tensor("b", [128, 128], F32, kind="ExternalOutput")
        with _tile.TileContext(nc) as tc:
            with tc.tile_pool(name="p", bufs=1) as p:
                t = p.tile([128, 128], F32)
                nc.sync.dma_start(t[:], a.ap())
                nc.sync.dma_start(b.ap(), t[:])
        nc.finalize()
        _CACHE["floor_nc"] = nc
    nc = _CACHE["floor_nc"]
    bass2jax.install_neuronx_cc_hook()
    partition_name = nc.partition_id_tensor.name if nc.partition_id_tensor else None
    in_names = ["a", "b"]
    if partition_name is not None:
        in_names.append(partition_name)
    out_avals = (jax.core.ShapedArray((128, 128), np.float32),)

    def _body(*args):
        ops = list(args)
        if partition_name is not None:
            ops = ops + [bass2jax.partition_id_tensor()]
        return tuple(_bass_exec_p.bind(
            *ops, out_avals=out_avals, in_names=tuple(in_names),
            out_names=("b",), lowering_input_output_aliases=(),
            sim_require_finite=True, sim_require_nnan=True, nc=nc))

    devices = jax.devices()[:NCORES]
    mesh = Mesh(np.asarray(devices), ("core",))
    f = jax.jit(shard_map(_body, mesh=mesh,
                          in_specs=(PartitionSpec("core"),) * 2,
                          out_specs=(PartitionSpec("core"),), check_rep=False))
    a = jax.device_put(np.zeros((NCORES * 128, 128), np.float32))
    z = jax.device_put(np.zeros((NCORES * 128, 128), np.float32))
    o = f(a, z); jax.block_until_ready(o)
    times = []
    for _ in range(reps):
        t0 = time.perf_counter()
        o = f(a, z)
        jax.block_until_ready(o)
        times.append(time.perf_counter() - t0)
    times.sort()
    return times[0] * 1e9


# revision 10
# speedup vs baseline: 4.9473x; 4.9473x over previous
"""Distributed Bass kernel for sparse cluster attention on 8 TRN2 NeuronCores.

Sharding: tensor-parallel over heads (16 heads -> 2 per core).
Per core:
  1. fp32-accurate keyframe q/k (hi/lo bf16 split) -> attn_score partial,
     AllReduce(max) over cores.
  2. main qkv in bf16: qT [ch,tok] in SBUF; k,v kept in SBUF as [tok,ch]
     tiles (no DRAM staging).
  3. on-device top-153 per cluster: rank via comparison matrix; selection
     materialized as per-cluster one-hot matrices [tok_chunk, rank] (bf16).
  4. gather k (-> [ch, j]) and v (-> packed [j, ch]) via one-hot matmuls on
     the tensor engine; flash-style attention (logits MM -> exp on ACT ->
     AV MM with ones-augmented v for the softmax denominator). Gathered
     blocks are packed 153*frames per source cluster, so every consumer
     cluster's kv list is a per-src prefix; partial chunks are handled by
     partition-range slices.
  5. AllToAll of per-core attention output -> proj on this core's token
     slice -> out [2048, 1024] f32; host concatenates.
"""

import numpy as np
import ml_dtypes

import os
import concourse.bass as bass
import concourse.bacc as bacc
import concourse.mybir as mybir
import concourse.tile as tile
from concourse.bass_utils import run_bass_kernel_spmd

BF16 = mybir.dt.bfloat16
F32 = mybir.dt.float32
I32 = mybir.dt.int32
AF = mybir.ActivationFunctionType
OP = mybir.AluOpType

# problem constants
H, D, C = 16, 64, 1024
S, P = 32, 512
K, FC = 4, 8
N = S * P                      # 16384 tokens
TK = 153                       # top-k patches per cluster
NSUB = 5                       # subsampled frames
NCORES = 8
HC = H // NCORES               # heads per core = 2
CHC = HC * D                   # channels per core = 128
TOKS = N // NCORES             # output tokens per core = 2048
SCALE = float(D) ** -0.5
KFT = K * P                    # keyframe tokens = 2048
FULL = FC * TK                 # packed kv rows per full src block = 1224
PRE5 = NSUB * TK               # packed kv rows per 5-frame prefix = 765
KGW = 1280                     # kg tile width (>= FULL, mult of 128)

_CACHE: dict = {}


def _chunks_for(ci):
    """(src, chunk, rows) list for consumer cluster ci over packed kv."""
    out = []
    for src in range(K):
        valid = FULL if src in (0, ci) else PRE5
        nch = (valid + 127) // 128
        for c in range(nch):
            out.append((src, c, min(128, valid - c * 128)))
    return out


def _win_frames(c):
    """Frames whose packed rows [f*153, (f+1)*153) intersect window
    [128c, 128(c+1))."""
    lo, hi = 128 * c, 128 * (c + 1)
    return [f for f in range(FC) if f * TK < hi and (f + 1) * TK > lo]


def build_nc(clusters, keyframes):
    NOAR = os.environ.get("KNOAR", "0") == "1"
    NOA2A = os.environ.get("KNOA2A", "0") == "1"
    STUB = os.environ.get("KSTUB", "0") == "1"
    nc = bacc.Bacc(None, target_bir_lowering=False, debug=False)

    # ---- kernel I/O (per-core shards prepared on host) ----
    xbT = nc.dram_tensor("xbT", [C, N], BF16, kind="ExternalInput")
    xkfT_l = nc.dram_tensor("xkfT_l", [C, KFT], BF16, kind="ExternalInput")
    wqkv = nc.dram_tensor("wqkv", [C, 3 * CHC], BF16, kind="ExternalInput")
    bqkv = nc.dram_tensor("bqkv", [3 * CHC], F32, kind="ExternalInput")
    wqk_h = nc.dram_tensor("wqk_h", [C, 2 * CHC], BF16, kind="ExternalInput")
    wqk_l = nc.dram_tensor("wqk_l", [C, 2 * CHC], BF16, kind="ExternalInput")
    bqk = nc.dram_tensor("bqk", [2 * CHC], F32, kind="ExternalInput")
    wproj = nc.dram_tensor("wproj", [C, C], BF16, kind="ExternalInput")
    bproj = nc.dram_tensor("bproj", [C], F32, kind="ExternalInput")
    out_ext = nc.dram_tensor("out", [TOKS, C], F32, kind="ExternalOutput")

    # ---- internal DRAM ----
    sc_in = nc.dram_tensor("sc_in", [K * P], F32)
    sc_out = nc.dram_tensor("sc_out", [K * P], F32, addr_space="Shared")
    ag_in = nc.dram_tensor("ag_in", [NCORES, CHC, TOKS], BF16)
    ag_out = nc.dram_tensor("ag_out", [NCORES, CHC, TOKS], BF16)

    if STUB:
        with tile.TileContext(nc) as tc:
            with tc.tile_pool(name="sp", bufs=2) as sp:
                t = sp.tile([128, 512], BF16)
                nc.sync.dma_start(t[:], xbT.ap()[0:128, 0:512])
                t2 = sp.tile([128, 512], F32)
                nc.vector.tensor_copy(t2[:], t[:])
                nc.sync.dma_start(out_ext.ap()[0:128, 0:512], t2[:])
        nc.finalize()
        return nc

    with tile.TileContext(nc) as tc:
        with (
            tc.tile_pool(name="persist", bufs=1) as pp,
            tc.tile_pool(name="work", bufs=3) as wp,
            tc.tile_pool(name="xp", bufs=10) as xp,
            tc.tile_pool(name="expw", bufs=2) as ep,
            tc.tile_pool(name="psmed", bufs=2, space="PSUM") as psM,
            tc.tile_pool(name="psav", bufs=2, space="PSUM") as psV,
            tc.tile_pool(name="psbig", bufs=1, space="PSUM") as psL,
        ):
            # ================= persistent SBUF =================
            qT = pp.tile([CHC, N], BF16, tag="qT")               # 4 MB
            k_sb = pp.tile([128, N // 128, CHC], BF16, tag="ksb")  # 4 MB
            v_sb = pp.tile([128, N // 128, CHC], BF16, tag="vsb")  # 4 MB
            kg = pp.tile([128, K, KGW], BF16, tag="kg")          # 1.25 MB
            vaug = pp.tile([128, K, 10, 130], BF16, tag="vaug")  # 1.3 MB
            ones_rowb = pp.tile([1, 128], BF16, tag="onesb")
            nc.vector.memset(ones_rowb[:], 1.0)
            onesf_row = pp.tile([1, 128], F32, tag="onesf")
            nc.vector.memset(onesf_row[:], 1.0)
            onesf_col = pp.tile([128, 1], F32, tag="onesfc")
            nc.vector.memset(onesf_col[:], 1.0)

            # weight tiles
            wqkv_t = pp.tile([128, 8, 3 * CHC], BF16, tag="wqkv")
            nc.sync.dma_start(wqkv_t[:], wqkv.ap().rearrange("(a p) c -> p a c", p=128))
            wqkh_t = pp.tile([128, 8, 2 * CHC], BF16, tag="wqkh")
            nc.sync.dma_start(wqkh_t[:], wqk_h.ap().rearrange("(a p) c -> p a c", p=128))
            wqkl_t = pp.tile([128, 8, 2 * CHC], BF16, tag="wqkl")
            nc.sync.dma_start(wqkl_t[:], wqk_l.ap().rearrange("(a p) c -> p a c", p=128))

            # bias columns (per-partition layout)
            bq_col = pp.tile([128, 1], F32, tag="bqcol")
            nc.sync.dma_start(bq_col[:], bqkv.ap()[0:CHC].rearrange("(p a) -> p a", a=1))
            bkv_row = pp.tile([1, 2 * CHC], F32, tag="bkvrow")
            nc.sync.dma_start(bkv_row[:], bqkv.ap()[CHC:3 * CHC].rearrange("(a c) -> a c", a=1))
            bkv_row_b = pp.tile([1, 2 * CHC], BF16, tag="bkvrowb")
            nc.vector.tensor_copy(bkv_row_b[:], bkv_row[:])
            bqk_q = pp.tile([128, 1], F32, tag="bqkq")
            nc.sync.dma_start(bqk_q[:], bqk.ap()[0:CHC].rearrange("(p a) -> p a", a=1))
            bqk_k = pp.tile([128, 1], F32, tag="bqkk")
            nc.sync.dma_start(bqk_k[:], bqk.ap()[CHC:2 * CHC].rearrange("(p a) -> p a", a=1))

            # ================= phase 1: keyframe scores (fp32 accurate) ======
            for tt in range(KFT // 512):
                kf = int(keyframes[tt])
                xh = [xp.tile([128, 512], BF16, tag="xmain", name=f"xh{tt}_{i}") for i in range(8)]
                xl = [xp.tile([128, 512], BF16, tag="xmain", name=f"xl{tt}_{i}") for i in range(8)]
                for cc in range(8):
                    nc.sync.dma_start(xh[cc][:], xbT.ap()[cc * 128:(cc + 1) * 128, kf * 512:(kf + 1) * 512])
                    nc.sync.dma_start(xl[cc][:], xkfT_l.ap()[cc * 128:(cc + 1) * 128, tt * 512:(tt + 1) * 512])
                qtile = wp.tile([128, 512], F32, tag="qkf", bufs=1)
                prod = wp.tile([128, 512], F32, tag="pkf", bufs=2)
                for ot, bias in ((0, bqk_q), (1, bqk_k)):
                    ps = psM.tile([128, 512], F32, tag="med")
                    nmm = 8 * 3
                    i = 0
                    for cc in range(8):
                        w_h = wqkh_t[:, cc, ot * CHC:(ot + 1) * CHC]
                        w_l = wqkl_t[:, cc, ot * CHC:(ot + 1) * CHC]
                        for (wt, xt) in ((w_h, xh[cc]), (w_h, xl[cc]), (w_l, xh[cc])):
                            nc.tensor.matmul(ps[:], wt, xt[:], start=(i == 0), stop=(i == nmm - 1))
                            i += 1
                    if ot == 0:
                        nc.vector.tensor_scalar(qtile[:], ps[:], bias[:], None, OP.add)
                    else:
                        # prod <- (k + bias) * q
                        nc.vector.tensor_scalar(prod[:], ps[:], bias[:], None, OP.add)
                        nc.vector.tensor_tensor(prod[:], prod[:], qtile[:], OP.mult)
                ps0 = psM.tile([1, 512], F32, tag="med")
                ps1 = psM.tile([1, 512], F32, tag="med")
                nc.tensor.matmul(ps0[:], onesf_col[0:64, :], prod[0:64, :], start=True, stop=True)
                nc.tensor.matmul(ps1[:], onesf_col[64:128, :], prod[64:128, :], start=True, stop=True)
                s1sb = wp.tile([1, 512], F32, tag="s1sb", bufs=1)
                nc.vector.tensor_copy(s1sb[:], ps1[:])
                smax_t = wp.tile([1, 512], F32, tag="smax", bufs=2)
                nc.vector.tensor_tensor(smax_t[:], ps0[:], s1sb[:], OP.max)
                nc.sync.dma_start(
                    sc_in.ap()[tt * 512:(tt + 1) * 512].rearrange("(a c) -> a c", a=1),
                    smax_t[:])
            if NOAR:
                nc.sync.dma_start(sc_out.ap(), sc_in.ap())
            else:
                nc.gpsimd.collective_compute(
                    "AllReduce", OP.max,
                    replica_groups=[list(range(NCORES))],
                    ins=[sc_in.ap().opt()],
                    outs=[sc_out.ap().opt()],
                )

            # ================= phase 2: main qkv (bf16) =================
            for tt in range(N // 512):
                xt = [xp.tile([128, 512], BF16, tag="xmain", name=f"xt{tt}_{i}") for i in range(8)]
                for cc in range(8):
                    nc.sync.dma_start(xt[cc][:], xbT.ap()[cc * 128:(cc + 1) * 128, tt * 512:(tt + 1) * 512])
                # q: [ch, tok]
                psq = psM.tile([128, 512], F32, tag="med")
                for cc in range(8):
                    nc.tensor.matmul(psq[:], wqkv_t[:, cc, 0:CHC], xt[cc][:],
                                     start=(cc == 0), stop=(cc == 7))
                nc.vector.tensor_scalar(qT[:, tt * 512:(tt + 1) * 512], psq[:], bq_col[:], None, OP.add)
                # k,v: [tok, ch] kept in SBUF
                for sub in range(4):
                    pskv = psM.tile([128, 2 * CHC], F32, tag="med")
                    for cc in range(8):
                        nc.tensor.matmul(pskv[:], xt[cc][:, sub * 128:(sub + 1) * 128],
                                         wqkv_t[:, cc, CHC:3 * CHC],
                                         start=(cc == 0), stop=False)
                    nc.tensor.matmul(pskv[:], ones_rowb[:], bkv_row_b[:],
                                     start=False, stop=True)
                    nc.vector.tensor_copy(k_sb[:, tt * 4 + sub, :], pskv[:, 0:CHC])
                    nc.vector.tensor_copy(v_sb[:, tt * 4 + sub, :], pskv[:, CHC:2 * CHC])

            # ================= phase 3: top-k -> packed token-id rows ========
            iota160 = wp.tile([128, 160], I32, tag="io160", bufs=1)
            nc.gpsimd.iota(iota160[:], pattern=[[1, 160]], base=0, channel_multiplier=0)
            iota160f = pp.tile([128, 160], F32, tag="io160f")
            nc.vector.tensor_copy(iota160f[:], iota160[:])
            iota_pv = wp.tile([128, 4], I32, tag="iopv", bufs=1)
            nc.gpsimd.iota(iota_pv[:], pattern=[[128, 4]], base=0, channel_multiplier=1)
            iota_pvf = pp.tile([128, 4], F32, tag="iopvf")
            nc.vector.tensor_copy(iota_pvf[:], iota_pv[:])
            # global-token iota: iota_tc[p, tc] = 128*tc + p
            iota_tc = wp.tile([128, N // 128], I32, tag="iotc", bufs=1)
            nc.gpsimd.iota(iota_tc[:], pattern=[[128, N // 128]], base=0, channel_multiplier=1)
            iota_tcf = pp.tile([128, N // 128], F32, tag="iotcf")
            nc.vector.tensor_copy(iota_tcf[:], iota_tc[:])

            psel_rows = {}
            for cl in range(K):
                s_row = wp.tile([1, P], F32, tag="srow", bufs=1)
                nc.sync.dma_start(s_row[:], sc_out.ap()[cl * P:(cl + 1) * P].rearrange("(a c) -> a c", a=1))
                s_colT = wp.tile([128, 4], F32, tag="scolT", bufs=1)
                nc.sync.dma_start(
                    s_colT[:], sc_out.ap()[cl * P:(cl + 1) * P].rearrange("(a p) -> p a", p=128))
                ps_bc = psM.tile([128, P], F32, tag="med")
                nc.tensor.matmul(ps_bc[:], onesf_row[:], s_row[:], start=True, stop=True)
                s_bc = wp.tile([128, P], F32, tag="sbc", bufs=2)
                nc.vector.tensor_copy(s_bc[:], ps_bc[:])
                ps_row = psM.tile([1, 160], F32, tag="med")
                for pc in range(4):
                    gt = wp.tile([128, P], BF16, tag="gtm", bufs=2)
                    nc.vector.tensor_scalar(gt[:], s_bc[:], s_colT[:, pc:pc + 1], None, OP.is_gt)
                    rank = wp.tile([128, 1], F32, tag="rank", bufs=2)
                    nc.vector.reduce_sum(rank[:], gt[:], axis=mybir.AxisListType.X)
                    eqr = wp.tile([128, 160], F32, tag="eqr", bufs=2)
                    nc.vector.tensor_scalar(eqr[:], iota160f[:], rank[:], None, OP.is_equal)
                    nc.tensor.matmul(ps_row[:], iota_pvf[:, pc:pc + 1], eqr[:],
                                     start=(pc == 0), stop=(pc == 3))
                psel_row = pp.tile([1, 160], F32, tag=f"pselr{cl}")
                nc.vector.tensor_copy(psel_row[:], ps_row[:])
                psel_rows[cl] = psel_row

            # ================= phase 3b: one-hot matmul gathers ==============
            # row2[j] = global token id of packed row j (frame j//153, rank j%153);
            # windows of its 128-partition broadcast give one-hot matrices.
            for src in range(K):
                psB2 = wp.tile([128, KGW], F32, tag="psB2", bufs=1)
                nc.vector.memset(psB2[:, FULL:KGW], -1.0)
                for f8 in range(FC):
                    fr = int(clusters[src][f8])
                    ps_b = psM.tile([128, 512], F32, tag="med")
                    nc.tensor.matmul(ps_b[:, 0:160], onesf_row[:], psel_rows[src][:],
                                     start=True, stop=True)
                    nc.vector.tensor_scalar(psB2[:, f8 * TK:(f8 + 1) * TK],
                                            ps_b[:, 0:TK], float(fr * P), None, OP.add)
                for c in range(10):
                    psk = psM.tile([128, 512], F32, tag="med")
                    psv = psM.tile([128, 512], F32, tag="med")
                    tcs = []
                    for f8 in _win_frames(c):
                        fr = int(clusters[src][f8])
                        tcs.extend(fr * 4 + i for i in range(4))
                    for ti, tc_ in enumerate(tcs):
                        ohW = wp.tile([128, 128], BF16, tag="ohW", bufs=4,
                                      name=f"ohW{src}_{c}_{ti}")
                        nc.vector.tensor_scalar(ohW[:], psB2[:, c * 128:(c + 1) * 128],
                                                iota_tcf[:, tc_:tc_ + 1], None, OP.is_equal)
                        nc.tensor.matmul(psk[:, 0:128], k_sb[:, tc_, :], ohW[:],
                                         start=(ti == 0), stop=(ti == len(tcs) - 1))
                        nc.tensor.matmul(psv[:, 0:128], ohW[:], v_sb[:, tc_, :],
                                         start=(ti == 0), stop=(ti == len(tcs) - 1))
                    nc.vector.tensor_copy(kg[:, src, c * 128:(c + 1) * 128], psk[:, 0:128])
                    nc.vector.tensor_copy(vaug[:, src, c, 0:64], psv[:, 0:64])
                    nc.vector.tensor_copy(vaug[:, src, c, 65:129], psv[:, 64:CHC])
            nc.vector.memset(vaug[:, :, :, 64:65], 1.0)
            nc.vector.memset(vaug[:, :, :, 129:130], 1.0)

            # ================= phase 4: attention per cluster =================
            for ci in range(K):
                chunks = _chunks_for(ci)
                ngrp = len(chunks) // 2
                for qt in range(FC):
                    f_q = int(clusters[ci][qt])
                    qsl = slice(f_q * P, (f_q + 1) * P)
                    ps_av = [psV.tile([65, 512], F32, tag="av", name=f"psav{ci}_{qt}_{i}") for i in range(2)]
                    for g in range(ngrp):
                        ps_lg = psL.tile([128, 2048], F32, tag="big")
                        for k01 in range(2):
                            src, c, rows = chunks[g * 2 + k01]
                            for h in range(2):
                                nc.tensor.matmul(
                                    ps_lg[:, (k01 * 2 + h) * 512:(k01 * 2 + h + 1) * 512],
                                    kg[h * 64:(h + 1) * 64, src, c * 128:(c + 1) * 128],
                                    qT[h * 64:(h + 1) * 64, qsl],
                                    start=True, stop=True,
                                    tile_position=(h * 64, 0))
                        ew = ep.tile([128, 2048], BF16, tag="ew")
                        nc.scalar.activation(ew[:], ps_lg[:], AF.Exp, scale=SCALE)
                        for k01 in range(2):
                            src, c, rows = chunks[g * 2 + k01]
                            for h in range(2):
                                nc.tensor.matmul(
                                    ps_av[h][:],
                                    vaug[0:rows, src, c, h * 65:(h + 1) * 65],
                                    ew[0:rows, (k01 * 2 + h) * 512:(k01 * 2 + h + 1) * 512],
                                    start=(g == 0 and k01 == 0),
                                    stop=(g == ngrp - 1 and k01 == 1))
                    # normalize and ship straight to the AllToAll staging buffer
                    otile = ep.tile([128, 512], BF16, tag="ot")
                    for h in range(2):
                        rec = wp.tile([1, 512], F32, tag="rec", bufs=2)
                        nc.vector.reciprocal(rec[:], ps_av[h][64:65, :])
                        ps_bc2 = psM.tile([64, 512], F32, tag="med")
                        nc.tensor.matmul(ps_bc2[:], onesf_row[:, 0:64], rec[:],
                                         start=True, stop=True)
                        bc_sb = wp.tile([64, 512], F32, tag="bcsb", bufs=2)
                        nc.vector.tensor_copy(bc_sb[:], ps_bc2[:])
                        nc.vector.tensor_tensor(
                            otile[h * 64:(h + 1) * 64, :],
                            ps_av[h][0:64, :], bc_sb[:], OP.mult)
                    jcore = (f_q * P) // TOKS
                    toff = (f_q * P) % TOKS
                    nc.sync.dma_start(ag_in.ap()[jcore, :, toff:toff + 512], otile[:])

            # ================= phase 5: AllToAll + proj =================
            if NOA2A:
                nc.sync.dma_start(ag_out.ap(), ag_in.ap())
            else:
                nc.gpsimd.collective_compute(
                    "AllToAll", OP.bypass,
                    replica_groups=[list(range(NCORES))],
                    ins=[ag_in.ap().opt()],
                    outs=[ag_out.ap().opt()],
                )
            wpj = pp.tile([128, 8, C], BF16, tag="vsb", name="wpj")
            nc.sync.dma_start(wpj[:], wproj.ap().rearrange("(a p) c -> p a c", p=128))
            bpj_row = pp.tile([1, C], BF16, tag="bpj")
            bpj_f = wp.tile([1, C], F32, tag="bpjf", bufs=1)
            nc.sync.dma_start(bpj_f[:], bproj.ap().rearrange("(a c) -> a c", a=1))
            nc.vector.tensor_copy(bpj_row[:], bpj_f[:])
            atk2 = pp.tile([128, 8, TOKS], BF16, tag="ksb", name="atk2")
            nc.sync.dma_start(atk2[:], ag_out.ap().rearrange("j p t -> p j t"))
            for mt in range(TOKS // 128):
                for ntile in range(2):
                    nsl = slice(ntile * 512, (ntile + 1) * 512)
                    ps = psM.tile([128, 512], F32, tag="med")
                    for cc in range(8):
                        nc.tensor.matmul(ps[:], atk2[:, cc, mt * 128:(mt + 1) * 128],
                                         wpj[:, cc, nsl], start=(cc == 0), stop=False)
                    nc.tensor.matmul(ps[:], ones_rowb[:], bpj_row[:, nsl],
                                     start=False, stop=True)
                    ot = wp.tile([128, 512], F32, tag="otile", bufs=2)
                    nc.vector.tensor_copy(ot[:], ps[:])
                    nc.sync.dma_start(
                        out_ext.ap()[mt * 128:(mt + 1) * 128, nsl], ot[:])

    nc.finalize()
    return nc


def _host_prep(x, W_qkv, b_qkv, W_proj, b_proj, clusters, keyframes):
    bf = ml_dtypes.bfloat16
    x2 = np.ascontiguousarray(x.reshape(N, C))
    xbT = np.ascontiguousarray(x2.T.astype(bf))                       # [C, N]
    kf_tok = np.concatenate([np.arange(P, dtype=np.int64) + int(f) * P for f in keyframes])
    xkf = x2[kf_tok]                                                   # [2048, C] f32
    xkf_h = xkf.astype(bf)
    xkf_l = (xkf - xkf_h.astype(np.float32)).astype(bf)
    xkfT_l = np.ascontiguousarray(xkf_l.T)

    in_maps = []
    for core in range(NCORES):
        h0 = core * HC
        qcols = np.arange(h0 * D, (h0 + HC) * D)
        wq = W_qkv[:, qcols]
        wk = W_qkv[:, C + qcols]
        wv = W_qkv[:, 2 * C + qcols]
        wqkv_s = np.concatenate([wq, wk, wv], axis=1)                  # [C, 384]
        bq = b_qkv[qcols]
        bk = b_qkv[C + qcols]
        bv = b_qkv[2 * C + qcols]
        wqk = np.concatenate([wq, wk], axis=1)                         # [C, 256]
        wqk_hi = wqk.astype(bf)
        wqk_lo = (wqk - wqk_hi.astype(np.float32)).astype(bf)
        in_maps.append({
            "xbT": xbT,
            "xkfT_l": xkfT_l,
            "wqkv": np.ascontiguousarray(wqkv_s.astype(bf)),
            "bqkv": np.ascontiguousarray(np.concatenate([bq, bk, bv]).astype(np.float32)),
            "wqk_h": np.ascontiguousarray(wqk_hi),
            "wqk_l": np.ascontiguousarray(wqk_lo),
            "bqk": np.ascontiguousarray(np.concatenate([bq, bk]).astype(np.float32)),
            "wproj": np.ascontiguousarray(W_proj.astype(bf)),
            "bproj": np.ascontiguousarray(b_proj.astype(np.float32)),
        })
    return in_maps


def kernel(x, W_qkv, b_qkv, W_proj, b_proj, clusters, keyframes, **run_kwargs):
    x = np.asarray(x, dtype=np.float32)
    W_qkv = np.asarray(W_qkv, dtype=np.float32)
    b_qkv = np.asarray(b_qkv, dtype=np.float32)
    W_proj = np.asarray(W_proj, dtype=np.float32)
    b_proj = np.asarray(b_proj, dtype=np.float32)
    clusters = np.asarray(clusters, dtype=np.int32)
    keyframes = np.asarray(keyframes, dtype=np.int32)

    key = (clusters.tobytes(), keyframes.tobytes(),
           os.environ.get("KNOAR"), os.environ.get("KNOA2A"), os.environ.get("KSTUB"))
    if _CACHE.get("key") != key:
        _CACHE["nc"] = build_nc(clusters, keyframes)
        _CACHE["key"] = key
    nc = _CACHE["nc"]

    in_maps = _host_prep(x, W_qkv, b_qkv, W_proj, b_proj, clusters, keyframes)
    res = run_bass_kernel_spmd(nc, in_maps, core_ids=list(range(NCORES)), **run_kwargs)
    _CACHE["last_result"] = res
    outs = res.results
    full = np.concatenate([np.asarray(outs[c]["out"], dtype=np.float32) for c in range(NCORES)], axis=0)
    return full.reshape(1, N, C)


def bench(x, W_qkv, b_qkv, W_proj, b_proj, clusters, keyframes, iters=10, reps=5):
    """Steady-state on-device timing: times the best of `reps` calls."""
    import time
    import jax
    from jax.sharding import Mesh, PartitionSpec
    from jax.experimental.shard_map import shard_map
    from concourse import bass2jax
    from concourse.bass2jax import _bass_exec_p
    import concourse.mybir as _mb

    clusters = np.asarray(clusters, dtype=np.int32)
    keyframes = np.asarray(keyframes, dtype=np.int32)
    key = (clusters.tobytes(), keyframes.tobytes(),
           os.environ.get("KNOAR"), os.environ.get("KNOA2A"), os.environ.get("KSTUB"))
    if _CACHE.get("key") != key:
        _CACHE["nc"] = build_nc(clusters, keyframes)
        _CACHE["key"] = key
    nc = _CACHE["nc"]
    bass2jax.install_neuronx_cc_hook()

    in_maps = _host_prep(np.asarray(x, np.float32), np.asarray(W_qkv, np.float32),
                         np.asarray(b_qkv, np.float32), np.asarray(W_proj, np.float32),
                         np.asarray(b_proj, np.float32), clusters, keyframes)

    in_names, out_names, out_avals, zero_outs = [], [], [], []
    partition_name = nc.partition_id_tensor.name if nc.partition_id_tensor else None
    for alloc in nc.m.functions[0].allocations:
        if not isinstance(alloc, _mb.MemoryLocationSet):
            continue
        name = alloc.memorylocations[0].name
        if alloc.kind == "ExternalInput":
            if name != partition_name:
                in_names.append(name)
        elif alloc.kind == "ExternalOutput":
            out_names.append(name)
            shape = tuple(alloc.tensor_shape)
            dtype = _mb.dt.np(alloc.dtype)
            out_avals.append(jax.core.ShapedArray(shape, dtype))
            zero_outs.append(np.zeros(shape, dtype))
    n_params = len(in_names)
    all_in_names = list(in_names) + list(out_names)
    if partition_name is not None:
        all_in_names.append(partition_name)

    def _body(*args):
        ops = list(args)
        if partition_name is not None:
            ops = ops + [bass2jax.partition_id_tensor()]
        outs = _bass_exec_p.bind(
            *ops,
            out_avals=tuple(out_avals),
            in_names=tuple(all_in_names),
            out_names=tuple(out_names),
            lowering_input_output_aliases=(),
            sim_require_finite=True,
            sim_require_nnan=True,
            nc=nc,
        )
        return tuple(outs)

    devices = jax.devices()[:NCORES]
    mesh = Mesh(np.asarray(devices), ("core",))
    in_specs = (PartitionSpec("core"),) * (n_params + len(out_names))
    out_specs = (PartitionSpec("core"),) * len(out_names)
    f = jax.jit(shard_map(_body, mesh=mesh, in_specs=in_specs,
                          out_specs=out_specs, check_rep=False))
    concat_in = [np.concatenate([np.asarray(in_maps[c][n]) for c in range(NCORES)], axis=0)
                 for n in in_names]
    concat_zeros = [np.zeros((NCORES * z.shape[0], *z.shape[1:]), z.dtype) for z in zero_outs]
    args = [jax.device_put(a) for a in concat_in + concat_zeros]
    o = f(*args)
    jax.block_until_ready(o)
    times = []
    for _ in range(max(reps, 20)):
        t0 = time.perf_counter()
        o = f(*args)
        jax.block_until_ready(o)
        times.append(time.perf_counter() - t0)
    times.sort()
    return times[0] * 1e9, times


def bench_floor(reps=20):
    """Dispatch-floor: time a trivial 8-core NEFF (one 64KB copy)."""
    import time
    import jax
    from jax.sharding import Mesh, PartitionSpec
    from jax.experimental.shard_map import shard_map
    from concourse import bass2jax
    from concourse.bass2jax import _bass_exec_p
    import concourse.bacc as _bacc
    import concourse.tile as _tile

    if "floor_nc" not in _CACHE:
        nc = _bacc.Bacc(None, target_bir_lowering=False, debug=False)
        a = nc.dram_tensor("a", [128, 128], F32, kind="ExternalInput")
        b = nc.dram_tensor("b", [128, 128], F32, kind="ExternalOutput")
        with _tile.TileContext(nc) as tc:
            with tc.tile_pool(name="p", bufs=1) as p:
                t = p.tile([128, 128], F32)
                nc.sync.dma_start(t[:], a.ap())
                nc.sync.dma_start(b.ap(), t[:])
        nc.finalize()
        _CACHE["floor_nc"] = nc
    nc = _CACHE["floor_nc"]
    bass2jax.install_neuronx_cc_hook()
    partition_name = nc.partition_id_tensor.name if nc.partition_id_tensor else None
    in_names = ["a", "b"]
    if partition_name is not None:
        in_names.append(partition_name)
    out_avals = (jax.core.ShapedArray((128, 128), np.float32),)

    def _body(*args):
        ops = list(args)
        if partition_name is not None:
            ops = ops + [bass2jax.partition_id_tensor()]
        return tuple(_bass_exec_p.bind(
            *ops, out_avals=out_avals, in_names=tuple(in_names),
            out_names=("b",), lowering_input_output_aliases=(),
            sim_require_finite=True, sim_require_nnan=True, nc=nc))

    devices = jax.devices()[:NCORES]
    mesh = Mesh(np.asarray(devices), ("core",))
    f = jax.jit(shard_map(_body, mesh=mesh,
                          in_specs=(PartitionSpec("core"),) * 2,
                          out_specs=(PartitionSpec("core"),), check_rep=False))
    a = jax.device_put(np.zeros((NCORES * 128, 128), np.float32))
    z = jax.device_put(np.zeros((NCORES * 128, 128), np.float32))
    o = f(a, z); jax.block_until_ready(o)
    times = []
    for _ in range(reps):
        t0 = time.perf_counter()
        o = f(a, z)
        jax.block_until_ready(o)
        times.append(time.perf_counter() - t0)
    times.sort()
    return times[0] * 1e9
